# revision 2
# baseline (speedup 1.0000x reference)
"""Cumulative LayerNorm (B=4, C=512, T=32000) on 8 TRN2 cores, v3.

Baseline architecture (core j = sample j//2, T-half j%2; h=1 cores
recompute the first-half totals with a flag-gated prefix pass — no
cross-core traffic) with three upgrades measured in earlier sessions:
  - bf16 input and output (host converts): read DMA halves, all
    fp32->bf16 convert ops disappear; rel err ~7e-3 << 2e-2 budget
  - channel-block pre-reduction (4 planes -> 2 on Pool/DVE): stats
    matmul columns halve
  - t-major cumulative sum: per-partition scans on [128, F] + a [2,128]
    cross-partition scan via tiny DRAM transpose round-trips, replacing
    the serial row scans (~2.6 ns/element on DVE)
Per-segment emission is software-pipelined (EARLY: load/squares/
pre-adds/stats/drain; LATE: t-major cumsum/finalize/broadcast/normalize/
store) so every engine has ready work queued ahead of the DRAM
round-trip tail.
"""
import numpy as np
import ml_dtypes

import concourse.bass as bass
import concourse.bacc as bacc
import concourse.tile as tile
from concourse import mybir
from concourse.bass_utils import run_bass_kernel_spmd

F32 = mybir.dt.float32
BF16 = mybir.dt.bfloat16
BF16_NP = ml_dtypes.bfloat16

B, C, T = 4, 512, 32000
NCORES = 8
TH = T // 2          # 16000 per core
CB = C // 128        # 4 channel blocks
SEG = 3200
NSEG = TH // SEG     # 5
F = SEG // 128       # 25
TS = 400             # stats chunk
NCH = SEG // TS      # 8 chunks -> st rows [16, TS]
TN = 400             # normalize / broadcast block
QS = 800             # square quarter
NQ = SEG // QS       # 4
EPS = 1e-08

_CACHE = {}


def _build(wb_general: bool):
    nc = bacc.Bacc()

    xc_e = nc.declare_dram_parameter("xc", [C, TH], BF16, isOutput=False)
    xp_e = nc.declare_dram_parameter("xp", [C, TH], BF16, isOutput=False)
    flag_e = nc.declare_dram_parameter("flag", [1, 1], F32, isOutput=False)
    invb_e = nc.declare_dram_parameter("invb", [128, NSEG * 2 * F], F32, isOutput=False)
    w_e = nc.declare_dram_parameter("w", [1, C], F32, isOutput=False)
    b_e = nc.declare_dram_parameter("b", [1, C], F32, isOutput=False)
    y_e = nc.declare_dram_parameter("y", [C, TH], BF16, isOutput=True)

    xc_r = xc_e.rearrange("(cb p) t -> cb p t", p=128)
    xp_r = xp_e.rearrange("(cb p) t -> cb p t", p=128)
    y_r = y_e.rearrange("(cb p) t -> cb p t", p=128)
    invb_r = invb_e.rearrange("p (s r f) -> p s r f", s=NSEG, r=2)

    with tile.TileContext(nc) as tc:
        with (
            tc.tile_pool(name="misc", bufs=1) as misc,
            tc.tile_pool(name="xin", bufs=3) as xin,
            tc.tile_pool(name="ztp", bufs=2) as ztp,
            tc.tile_pool(name="xz2", bufs=2) as xz2p,
            tc.tile_pool(name="tmaj", bufs=2) as tmaj,
            tc.tile_pool(name="abr", bufs=2) as abr,
            tc.tile_pool(name="dram", bufs=3, space="DRAM") as dram,
            tc.tile_pool(name="pstat", bufs=2, space="PSUM") as pstat,
            tc.tile_pool(name="pab", bufs=2, space="PSUM") as pab,
        ):
            # ---- constants
            # stats lhsT: chunk c lands on st partitions 2c (s) / 2c+1 (q)
            es_t = misc.tile([128, NCH, 2 * NCH], BF16, tag="es_t")
            eq_t = misc.tile([128, NCH, 2 * NCH], BF16, tag="eq_t")
            nc.vector.memset(es_t, 0.0)
            nc.vector.memset(eq_t, 0.0)
            for c in range(NCH):
                nc.vector.memset(es_t[:, c, 2 * c : 2 * c + 1], 1.0)
                nc.vector.memset(eq_t[:, c, 2 * c + 1 : 2 * c + 2], 1.0)
            ones1 = misc.tile([128, 1], BF16, tag="ones1")
            nc.vector.memset(ones1, 1.0)
            ones1f = misc.tile([128, 1], F32, tag="ones1f")
            nc.vector.memset(ones1f, 1.0)
            ones_bf = misc.tile([1, 128], BF16, tag="ones_bf")
            nc.vector.memset(ones_bf, 1.0)
            eps_t = misc.tile([128, 1], F32, tag="eps_t")
            nc.vector.memset(eps_t, EPS)
            zer = misc.tile([128, 128], F32, tag="zer")
            nc.vector.memset(zer, 0.0)
            carryT = misc.tile([2, 1], F32, tag="carryT")
            flag_t = misc.tile([1, 1], F32, tag="flag_t")
            nc.sync.dma_start(out=flag_t, in_=flag_e[:, :])
            invb_t = misc.tile([128, NSEG, 2, F], F32, tag="invb_t")
            nc.sync.dma_start(out=invb_t, in_=invb_r)
            if wb_general:
                wcol = misc.tile([128, CB], F32, tag="wcol")
                bcol = misc.tile([128, CB], F32, tag="bcol")
                for cb in range(CB):
                    nc.sync.dma_start(
                        out=wcol[:, cb : cb + 1],
                        in_=w_e[0:1, cb * 128 : (cb + 1) * 128].rearrange(
                            "one p -> (one p) 1"
                        ),
                    )
                    nc.sync.dma_start(
                        out=bcol[:, cb : cb + 1],
                        in_=b_e[0:1, cb * 128 : (cb + 1) * 128].rearrange(
                            "one p -> (one p) 1"
                        ),
                    )
            else:
                wdummy = misc.tile([1, C], F32, tag="wdummy")
                nc.sync.dma_start(out=wdummy, in_=w_e[:, :])
                nc.sync.dma_start(out=wdummy, in_=b_e[:, :])

            # ---- prefix reduce phase: totals of xp, flag-gated -> carryT
            NPQ = NSEG * NQ
            q_acc = misc.tile([128, NPQ], F32, tag="q_acc")
            with tc.tile_pool(name="ppre", bufs=1, space="PSUM") as pre_ps:
                tot_s = pre_ps.tile([1, 512], F32, tag="tot_s")
                for s in range(NSEG):
                    xt = xin.tile([128, CB, SEG], BF16, tag="x")
                    for cb in range(CB):
                        nc.sync.dma_start(
                            out=xt[:, cb, :],
                            in_=xp_r[cb, :, s * SEG : (s + 1) * SEG],
                        )
                    x2p = xz2p.tile([128, 2, SEG], BF16, tag="x2p")
                    for j in range(2):
                        nc.gpsimd.tensor_add(
                            out=x2p[:, j, :], in0=xt[:, 2 * j, :],
                            in1=xt[:, 2 * j + 1, :],
                        )
                    for q in range(NQ):
                        i = s * NQ + q
                        zq = ztp.tile([128, CB, QS], BF16, tag="zq")
                        nc.scalar.activation(
                            out=zq,
                            in_=xt[:, :, q * QS : (q + 1) * QS],
                            func=mybir.ActivationFunctionType.Square,
                            accum_out=q_acc[:, i : i + 1],
                        )
                    for c in range(2 * NCH):
                        j, cc = c % 2, c // 2
                        cs = slice(cc * TS, (cc + 1) * TS)
                        nc.tensor.matmul(
                            out=tot_s[:, 0:TS], lhsT=ones1, rhs=x2p[:, j, cs],
                            start=(s == 0 and c == 0),
                            stop=(s == NSEG - 1 and c == 2 * NCH - 1),
                        )
                tot_q = pre_ps.tile([1, NPQ], F32, tag="tot_q")
                nc.tensor.matmul(
                    out=tot_q, lhsT=ones1f, rhs=q_acc, start=True, stop=True
                )
                sred = misc.tile([1, 1], F32, tag="sred")
                qred = misc.tile([1, 1], F32, tag="qred")
                nc.vector.reduce_sum(
                    out=sred, in_=tot_s[:, 0:TS], axis=mybir.AxisListType.X
                )
                nc.vector.reduce_sum(out=qred, in_=tot_q, axis=mybir.AxisListType.X)
                cgate = misc.tile([1, 2], F32, tag="cgate")
                nc.vector.tensor_mul(out=cgate[:, 0:1], in0=sred, in1=flag_t)
                nc.vector.tensor_mul(out=cgate[:, 1:2], in0=qred, in1=flag_t)
                d_c = dram.tile([2, 1], F32, tag="d_c")
                nc.sync.dma_start(
                    out=d_c.rearrange("r one -> one r"), in_=cgate
                )
                nc.sync.dma_start(out=carryT, in_=d_c[:, :])

            # ---- main phase: software-pipelined early/late
            def early(s):
                xt = xin.tile([128, CB, SEG], BF16, tag="x")
                for cb in range(CB):
                    nc.sync.dma_start(
                        out=xt[:, cb, :], in_=xc_r[cb, :, s * SEG : (s + 1) * SEG]
                    )
                x2z2 = xz2p.tile([128, 4, SEG], BF16, tag="x2z2")
                for j in range(2):
                    nc.gpsimd.tensor_add(
                        out=x2z2[:, j, :], in0=xt[:, 2 * j, :],
                        in1=xt[:, 2 * j + 1, :],
                    )
                for q in range(NQ):
                    qs = slice(q * QS, (q + 1) * QS)
                    zq = ztp.tile([128, CB, QS], BF16, tag="zq")
                    nc.scalar.activation(
                        out=zq,
                        in_=xt[:, :, qs],
                        func=mybir.ActivationFunctionType.Square,
                    )
                    for j in range(2):
                        nc.vector.tensor_add(
                            out=x2z2[:, 2 + j, qs], in0=zq[:, 2 * j, :],
                            in1=zq[:, 2 * j + 1, :],
                        )
                st = pstat.tile([2 * NCH, 512], F32, tag="st")
                for c in range(NCH):
                    cs = slice(c * TS, (c + 1) * TS)
                    for j in range(2):
                        nc.tensor.matmul(
                            out=st[:, 0:TS], lhsT=es_t[:, c, :],
                            rhs=x2z2[:, j, cs],
                            start=(c == 0 and j == 0), stop=False,
                        )
                    for j in range(2):
                        nc.tensor.matmul(
                            out=st[:, 0:TS], lhsT=eq_t[:, c, :],
                            rhs=x2z2[:, 2 + j, cs],
                            start=False, stop=(c == NCH - 1 and j == 1),
                        )
                stat_sb = abr.tile([2 * NCH, TS], F32, tag="stat_sb")
                nc.scalar.copy(out=stat_sb, in_=st[:, 0:TS])
                d_st = dram.tile([2 * NCH, TS], F32, tag="d_st")
                nc.scalar.dma_start(out=d_st[:, :], in_=stat_sb)
                return xt, d_st

            def late(s, xt, d_st):
                tm2 = tmaj.tile([128, 2, F], F32, tag="tm2")
                for r in range(2):
                    src = bass.AP(
                        tensor=d_st.tensor,
                        offset=d_st.offset + r * TS,
                        ap=[[2 * TS, 128 * F // TS], [F, TS // F], [1, F]],
                    )
                    nc.scalar.dma_start(out=tm2[:, r, :], in_=src)
                for r in range(2):
                    nc.vector.tensor_tensor_scan(
                        out=tm2[:, r, :], data0=tm2[:, r, :], data1=zer[:, 0:F],
                        initial=0.0, op0=mybir.AluOpType.add,
                        op1=mybir.AluOpType.bypass,
                    )
                d_pt = dram.tile([128, 2], F32, tag="d_pt")
                nc.scalar.dma_start(out=d_pt[:, :], in_=tm2[:, :, F - 1 : F])
                ptT = tmaj.tile([2, 128], F32, tag="ptT")
                nc.scalar.dma_start(out=ptT, in_=d_pt.rearrange("p r -> r p"))
                scT = tmaj.tile([2, 128], F32, tag="scT")
                nc.vector.tensor_tensor_scan(
                    out=scT, data0=ptT, data1=zer[0:2, 0:128],
                    initial=carryT, op0=mybir.AluOpType.add,
                    op1=mybir.AluOpType.bypass,
                )
                nc.vector.tensor_copy(out=carryT, in_=scT[:, 127:128])
                offsT = tmaj.tile([2, 128], F32, tag="offsT")
                nc.vector.tensor_sub(out=offsT, in0=scT, in1=ptT)
                d_off = dram.tile([2, 128], F32, tag="d_off")
                nc.scalar.dma_start(out=d_off[:, :], in_=offsT)
                offs = tmaj.tile([128, 2], F32, tag="offs")
                nc.scalar.dma_start(out=offs, in_=d_off.rearrange("r p -> p r"))
                rep_off = bass.AP(
                    tensor=offs.tensor, offset=offs.offset,
                    ap=[offs.ap[0], offs.ap[1], [0, F]],
                )
                nc.vector.tensor_add(out=tm2, in0=tm2, in1=rep_off)

                vv = tmaj.tile([128, 2, F], F32, tag="vv")
                nc.vector.tensor_mul(out=vv, in0=tm2, in1=invb_t[:, s, :, :])
                msq = tmaj.tile([128, F], F32, tag="msq")
                nc.gpsimd.tensor_mul(out=msq, in0=vv[:, 0, :], in1=vv[:, 0, :])
                var = tmaj.tile([128, F], F32, tag="var")
                nc.gpsimd.tensor_sub(out=var, in0=vv[:, 1, :], in1=msq)
                nc.gpsimd.tensor_scalar_max(out=var, in0=var, scalar1=0.0)
                sd = tmaj.tile([128, F], F32, tag="sd")
                nc.scalar.activation(
                    out=sd, in_=var, func=mybir.ActivationFunctionType.Sqrt,
                    bias=eps_t, scale=1.0,
                )
                tmo = tmaj.tile([128, 2, F], BF16, tag="tmo")
                with nc.allow_low_precision(
                    reason="bf16 A/B rows feed PE broadcast matmuls"
                ):
                    nc.vector.reciprocal(out=tmo[:, 0, :], in_=sd)
                    nc.vector.tensor_mul(
                        out=tmo[:, 1, :], in0=vv[:, 0, :], in1=tmo[:, 0, :]
                    )
                d_ab = dram.tile([2, SEG], BF16, tag="d_ab")
                nc.scalar.dma_start(
                    out=d_ab.rearrange("r (p f) -> p r f", p=128), in_=tmo
                )
                arow = abr.tile([1, SEG], BF16, tag="arow")
                brow = abr.tile([1, SEG], BF16, tag="brow")
                nc.scalar.dma_start(out=arow, in_=d_ab[0:1, :])
                nc.scalar.dma_start(out=brow, in_=d_ab[1:2, :])

                for j in range(SEG // TN):
                    js = slice(j * TN, (j + 1) * TN)
                    ab = pab.tile([128, 2, 512], F32, tag="ab")
                    nc.tensor.matmul(
                        out=ab[:, 0, 0:TN], lhsT=ones_bf, rhs=arow[0:1, js],
                        start=True, stop=True,
                    )
                    nc.tensor.matmul(
                        out=ab[:, 1, 0:TN], lhsT=ones_bf, rhs=brow[0:1, js],
                        start=True, stop=True,
                    )
                    pa = ab[:, 0, 0:TN]
                    pb = ab[:, 1, 0:TN]
                    rep_a = bass.AP(
                        tensor=pa.tensor, offset=pa.offset,
                        ap=[pa.ap[0], [0, CB], pa.ap[1]],
                    )
                    rep_b = bass.AP(
                        tensor=pb.tensor, offset=pb.offset,
                        ap=[pb.ap[0], [0, CB], pb.ap[1]],
                    )
                    xs = xt[:, :, js]
                    nc.vector.tensor_mul(out=xs, in0=xs, in1=rep_a)
                    nc.vector.tensor_add(out=xs, in0=xs, in1=rep_b)
                    if wb_general:
                        for cb in range(CB):
                            nc.scalar.activation(
                                out=xs[:, cb, :], in_=xs[:, cb, :],
                                func=mybir.ActivationFunctionType.Copy,
                                bias=0.0, scale=wcol[:, cb : cb + 1],
                            )
                            nc.vector.tensor_scalar_add(
                                out=xs[:, cb, :], in0=xs[:, cb, :],
                                scalar1=bcol[:, cb : cb + 1],
                            )
                for cb in range(CB):
                    nc.sync.dma_start(
                        out=y_r[cb, :, s * SEG : (s + 1) * SEG], in_=xt[:, cb, :]
                    )

            pend = None
            for s in range(NSEG + 1):
                cur = early(s) if s < NSEG else None
                if pend is not None:
                    late(s - 1, *pend)
                pend = cur

    nc.finalize()
    return nc


def _get_kernel(wb_general: bool):
    if wb_general not in _CACHE:
        _CACHE[wb_general] = _build(wb_general)
    return _CACHE[wb_general]


def _make_in_maps(x, weight, bias):
    wb_general = not (np.all(weight == 1.0) and np.all(bias == 0.0))
    w_row = np.ascontiguousarray(weight.reshape(1, C).astype(np.float32))
    b_row = np.ascontiguousarray(bias.reshape(1, C).astype(np.float32))
    xbf = np.asarray(x, np.float32).astype(BF16_NP)
    in_maps = []
    for core in range(NCORES):
        b_idx, h = core // 2, core % 2
        xc = np.ascontiguousarray(xbf[b_idx, :, h * TH : (h + 1) * TH])
        xp = np.ascontiguousarray(xbf[b_idx, :, 0:TH]) if h == 1 else xc
        flag = np.full((1, 1), float(h), np.float32)
        # invb[p, s, 0, f] = -1/(C*(t+1)); [.., 1, .] = +1/(C*(t+1));
        # t global = h*TH + s*SEG + p*F + f
        t_l = (
            np.arange(NSEG)[:, None, None] * SEG
            + np.arange(128)[None, :, None] * F
            + np.arange(F)[None, None, :]
        )
        t_g = h * TH + t_l
        invn = (1.0 / (C * (t_g.astype(np.float64) + 1.0))).astype(np.float32)
        invn = invn.transpose(1, 0, 2)  # [128, NSEG, F]
        invb = np.stack([-invn, invn], axis=2)  # [128, NSEG, 2, F]
        invb = np.ascontiguousarray(invb.reshape(128, NSEG * 2 * F))
        in_maps.append(
            {
                "xc": xc, "xp": xp, "flag": flag, "invb": invb,
                "w": w_row, "b": b_row,
            }
        )
    return in_maps, wb_general


def kernel(x, weight, bias, _trace=False, _tmpdir=None):
    x = np.asarray(x, np.float32)
    weight = np.asarray(weight, np.float32)
    bias = np.asarray(bias, np.float32)
    in_maps, wb_general = _make_in_maps(x, weight, bias)
    nc = _get_kernel(wb_general)
    res = run_bass_kernel_spmd(
        nc, in_maps, list(range(NCORES)), trace=_trace, tmpdir=_tmpdir
    )
    y = np.empty((B, C, T), np.float32)
    for core in range(NCORES):
        b_idx, h = core // 2, core % 2
        y[b_idx, :, h * TH : (h + 1) * TH] = res.results[core]["y"].astype(
            np.float32
        )
    if _trace:
        return y, res
    return y
